# revision 1
# baseline (speedup 1.0000x reference)
"""Trainium2 Bass kernel for single-query cross-attention pooling
(segment softmax over equal-sized node segments, 8-way node/graph sharded).

Self-contained: hardcodes N=262144, D=256, H=8, G=1024, 8 cores.

Algorithm (per core, 128 graphs / 32768 nodes, all device math in fp16 with
fp32 PSUM accumulation):
  scores[n,h] = x[n,:] @ A2[:,h]        A2 = scale * Wk_h^T q_h  (host-folded;
                                        per-head additive consts cancel in the
                                        segment softmax, so they are dropped)
  p = exp(scores - U)                   U = uniform host bound, cancels too
  Z[g,h,:] = sum_{n in g} p[n,h]*[x[n,:] | 1]   (pool BEFORE projecting:
                                        projection commutes with the pooled sum)
  out[g,:] = sum_h (Z[g,h,:256]/Z[g,h,256]) @ CH_h + bout
                                        CH_h = Wv_h^T Wout_h^T  (host-folded)

The transposed copy of x (xT) is prepared on the host so scores can be computed
with x-tiles as the matmul stationary operand -> node-major scores, and no
on-device transposes of x are ever needed.  Total HBM traffic per core is
2 * 16 MB fp16 = the fp32-read-once roofline.
"""

import numpy as np

N_TOTAL = 262144
D = 256
H = 8
DH = 32
G_TOTAL = 1024
N_CORES = 8

N_LOC = N_TOTAL // N_CORES          # 32768 nodes per core
G_LOC = G_TOTAL // N_CORES          # 128 graphs per core
SEG = N_TOTAL // G_TOTAL            # 256 nodes per graph (equal segments)
NT = N_LOC // 128                   # 256 node tiles per core

_CACHE = {}


def _build_program(U, repeats=1, loop_iters=0, skip=(), tiny_out=False,
                   xnch=16, xtch=32):
    """skip: subset of {'scores','z','xt_dma','xn_dma'} - diagnostic variants
    that elide parts of the per-tile work (output garbage; timing only).
    tiny_out: shrink the output tensor to [1,4] so relay transport per call
    is negligible (timing variants)."""
    import concourse.bass as bass
    import concourse.tile as tile
    from concourse import bacc, mybir
    from contextlib import ExitStack

    f16 = mybir.dt.float16
    f32 = mybir.dt.float32

    nc = bacc.Bacc("TRN2", target_bir_lowering=False, debug=False,
                   num_devices=N_CORES)

    # x_nat is host-pre-tiled: [chunk, partition, tile_in_chunk * (D+1)] so a
    # chunk DMA is 128 contiguous 8224-byte descriptors instead of 2048
    # small ones (HWDGE descriptor-generation was the SP-seq bottleneck).
    XNCH = xnch          # x_nat tiles per DMA chunk
    XTCH = xtch          # xT tiles per DMA chunk
    x_nat = nc.dram_tensor("x_nat", [NT // XNCH, 128, XNCH * (D + 1)], f16,
                           kind="ExternalInput")
    x_t = nc.dram_tensor("x_t", [D, N_LOC], f16, kind="ExternalInput")
    a2_d = nc.dram_tensor("a2", [D, 128], f16, kind="ExternalInput")
    ch_d = nc.dram_tensor("ch", [128, 32 * 128], f16, kind="ExternalInput")
    bout_d = nc.dram_tensor("bout", [128, 2], f32, kind="ExternalInput")
    ebias_d = nc.dram_tensor("ebias", [128, 2], f32, kind="ExternalInput")
    id16_d = nc.dram_tensor("id16", [128, 128], f16, kind="ExternalInput")
    id32_d = nc.dram_tensor("id32", [128, 128], f32, kind="ExternalInput")
    out_shape = [1, 4] if tiny_out else [G_LOC, D]
    out_d = nc.dram_tensor("out", out_shape, f32, kind="ExternalOutput")

    Exp = mybir.ActivationFunctionType.Exp
    Copy = mybir.ActivationFunctionType.Copy
    Ident = mybir.ActivationFunctionType.Identity

    with tile.TileContext(nc) as tc, ExitStack() as ctx:
        consts = ctx.enter_context(tc.tile_pool(name="consts", bufs=1))
        xn_pool = ctx.enter_context(tc.tile_pool(name="xn_pool", bufs=8))
        xt_pool = ctx.enter_context(tc.tile_pool(name="xt_pool", bufs=6))
        zn_pool = ctx.enter_context(tc.tile_pool(name="zn_pool", bufs=2))
        p_pool = ctx.enter_context(tc.tile_pool(name="p_pool", bufs=3))
        small = ctx.enter_context(tc.tile_pool(name="small", bufs=8))
        tailsb = ctx.enter_context(tc.tile_pool(name="tailsb", bufs=2))
        # PSUM: 8 banks total; tags share slots across main loop and tail
        bigps = ctx.enter_context(
            tc.tile_pool(name="bigps", bufs=2, space="PSUM"))
        smallps = ctx.enter_context(
            tc.tile_pool(name="smallps", bufs=2, space="PSUM"))
        zpsum = ctx.enter_context(
            tc.tile_pool(name="zpsum", bufs=2, space="PSUM"))
        apsum = ctx.enter_context(
            tc.tile_pool(name="apsum", bufs=2, space="PSUM"))

        # ---- constants ----
        # a2 here is "a2pad": 4 placement variants of the [c,8] score matrix,
        # variant w at cols [32w+8w_slot...]: A2 occupies cols 32w+8w..,
        # rest zeros (see _host_fold).  Slicing variant w gives a [128,32]
        # stationary whose zero columns make the scores land at the exact
        # psum-row slot the pooling matmul later needs -- so the transposed
        # p tile is directly usable as the pooling stationary.
        a2_0 = consts.tile([128, 128], f16)
        a2_1 = consts.tile([128, 128], f16)
        nc.gpsimd.dma_start(out=a2_0, in_=a2_d.ap()[0:128, :])
        nc.gpsimd.dma_start(out=a2_1, in_=a2_d.ap()[128:256, :])
        ch_sb = consts.tile([128, 32 * 128], f16)
        nc.gpsimd.dma_start(out=ch_sb, in_=ch_d.ap())
        bout_sb = consts.tile([128, 2], f32)
        nc.gpsimd.dma_start(out=bout_sb, in_=bout_d.ap())
        id16 = consts.tile([128, 128], f16)
        nc.gpsimd.dma_start(out=id16, in_=id16_d.ap())
        id32 = consts.tile([128, 128], f32)
        nc.gpsimd.dma_start(out=id32, in_=id32_d.ap())
        # exp bias: per-psum-row bias selecting live score slots (-U) vs dead
        # padding rows (-50 -> exp underflows to exactly 0 in fp16, so the
        # transposed p slices have true zeros outside each graph's slot).
        ebias = consts.tile([128, 2], f32)
        nc.gpsimd.dma_start(out=ebias, in_=ebias_d.ap())

        def emit_body():
            # ---- main streaming loop, pipelined in 4-tile half-groups ------
            # per half-group b (tiles 4b..4b+3 = 2 graphs):
            #   phase A: 8 score matmuls (a2pad stationary, M=32 at 32-aligned
            #            psum rows) -> sT128 [128,128] fully written, then ONE
            #            exp [128,128] -> p128 f16
            #   phase B (b-1): one [128,128] PE transpose; its output slices
            #            [:, 32j:32j+32] ARE the pooling stationaries (p at
            #            slot 8*(g%4), zeros elsewhere, from a2pad placement);
            #            4 Z-matmuls; drain every 8th half-group
            NB = NT // 4
            state = {}      # b -> (p128, xn_chunk)
            xn_chunk = xt0 = xt1 = zp = None
            znT = [tailsb.tile([128, 8 * 128], f16, name=f"znT{c}",
                               tag=f"znT{c}") for c in range(2)]

            def phase_a(b):
                nonlocal xn_chunk, xt0, xt1
                t0 = 4 * b
                for t in range(t0, t0 + 4):
                    if t % XNCH == 0 and "xn_dma" not in skip:
                        k = t // XNCH
                        xn_chunk = xn_pool.tile([128, XNCH, D + 1], f16,
                                                name="xn")
                        nc.scalar.dma_start(
                            out=xn_chunk.rearrange("p k c -> p (k c)"),
                            in_=x_nat.ap()[k])
                    if t % XTCH == 0 and "xt_dma" not in skip:
                        k = t // XTCH
                        sl = slice(k * XTCH * 128, (k + 1) * XTCH * 128)
                        xt0 = xt_pool.tile([128, XTCH * 128], f16, name="xt0")
                        xt1 = xt_pool.tile([128, XTCH * 128], f16, name="xt1")
                        nc.sync.dma_start(out=xt0, in_=x_t.ap()[0:128, sl])
                        nc.scalar.dma_start(out=xt1,
                                            in_=x_t.ap()[128:256, sl])
                state[b] = [None, xn_chunk]
                if "scores" in skip:
                    return
                sT = bigps.tile([128, 128], f32, name="sT", tag="bigps")
                for j in range(4):
                    t = t0 + j
                    w = (2 * b + j // 2) % 4
                    lo = (t % XTCH) * 128
                    for ch, a2c, xtc in ((0, a2_0, xt0), (1, a2_1, xt1)):
                        nc.tensor.matmul(
                            sT[32 * j:32 * j + 32, :],
                            lhsT=a2c[:, 32 * w:32 * w + 32],
                            rhs=xtc[:, lo:lo + 128],
                            start=(ch == 0), stop=(ch == 1),
                            tile_position=(0, 32 * j))
                p128 = p_pool.tile([128, 128], f16, name="p128")
                nc.scalar.activation(out=p128, in_=sT, func=Exp,
                                     bias=ebias[:, b % 2:b % 2 + 1],
                                     scale=1.0)
                state[b][0] = p128

            def phase_b(b):
                nonlocal zp
                p128, xn_c = state.pop(b)
                if "scores" not in skip:
                    tp = smallps.tile([128, 128], f16, name="tp", tag="tp")
                    nc.tensor.transpose(tp, p128, id16)
                    pT = p_pool.tile([128, 128], f16, name="pT", tag="pT")
                    nc.vector.tensor_copy(pT, tp)
                for j in range(4):
                    t = 4 * b + j
                    if t % 32 == 0 and not set(skip) & {"scores", "z"}:
                        zp = zpsum.tile([128, D + 1], f32, name="zp")
                    strip = (t % 32) // 8
                    if "z" not in skip and "scores" not in skip:
                        nc.tensor.matmul(zp[32 * strip:32 * strip + 32, :],
                                         lhsT=pT[:, 32 * j:32 * j + 32],
                                         rhs=xn_c[:, t % XNCH, :],
                                         start=(t % 8 == 0),
                                         stop=(t % 8 == 7),
                                         tile_position=(0, 32 * strip))
                if set(skip) & {"scores", "z"}:
                    return
                if (4 * b) % 32 == 28:  # last half-group of a psum window
                    k = (4 * b) // 32
                    rd = small.tile([128, 1], f32, name="rd")
                    nc.vector.reciprocal(rd, zp[:, D:D + 1])
                    zn = zn_pool.tile([128, D], f16, name="zn")
                    nc.scalar.activation(out=zn, in_=zp[:, 0:D], func=Copy,
                                         bias=0.0, scale=rd)
                    # transpose this window's Zn right away so only the CH
                    # projection remains after the streaming loop
                    for c in range(2):
                        tpz = smallps.tile([128, 128], f16, name="tpz",
                                           tag="tp")
                        nc.tensor.transpose(
                            tpz, zn[:, 128 * c:128 * (c + 1)], id16)
                        nc.vector.tensor_copy(
                            znT[c][:, 128 * k:128 * (k + 1)], tpz)

            for b in range(NB + 1):
                if b < NB:
                    phase_a(b)
                if b >= 1:
                    phase_b(b - 1)

            if set(skip) & {"scores", "z"}:
                # diagnostic variant: no drains/tail; emit a token output DMA
                nc.sync.dma_start(out=out_d.ap()[0:1, 0:4],
                                  in_=id32[0:1, 0:4])
                return

            # ---- tail: project with folded CH, add bias ----
            outsb = tailsb.tile([128, D], f32, name="outsb")
            for dh in range(2):
                ap_ps = apsum.tile([128, 128], f32, name="ap_ps")
                for h in range(H):
                    for c in range(2):
                        w = ch_sb[:, ((h * 2 + c) * 2 + dh) * 128:
                                  ((h * 2 + c) * 2 + dh) * 128 + 128]
                        rhs = znT[c].rearrange("p (k g h) -> p h (k g)",
                                               g=16, h=8)[:, h, :]
                        nc.tensor.matmul(ap_ps, lhsT=w, rhs=rhs,
                                         start=(h == 0 and c == 0),
                                         stop=(h == H - 1 and c == 1))
                atb = tailsb.tile([128, 128], f32, name=f"atb{dh}")
                nc.scalar.activation(out=atb, in_=ap_ps, func=Ident,
                                     bias=bout_sb[:, dh:dh + 1], scale=1.0)
                tp2 = apsum.tile([128, 128], f32, name="ap_ps")
                nc.tensor.transpose(tp2, atb, id32)
                nc.vector.tensor_copy(outsb[:, 128 * dh:128 * (dh + 1)], tp2)

            if tiny_out:
                nc.sync.dma_start(out=out_d.ap(), in_=outsb[0:1, 0:4])
            else:
                nc.sync.dma_start(out=out_d.ap(), in_=outsb)

        if loop_iters:
            with tc.For_i(0, loop_iters, 1):
                emit_body()
        else:
            for _rep in range(repeats):
                emit_body()

    nc.compile()
    return nc


def _host_fold(query, in_proj_weight, in_proj_bias, out_proj_weight,
               out_proj_bias):
    W = np.asarray(in_proj_weight, np.float64)
    Wq, Wk, Wv = W[:D], W[D:2 * D], W[2 * D:]
    b = np.asarray(in_proj_bias, np.float64)
    bq = b[:D]
    Wout = np.asarray(out_proj_weight, np.float64)
    scale = 1.0 / np.sqrt(DH)
    q = (np.asarray(query, np.float64).reshape(D) @ Wq.T + bq).reshape(H, DH)
    A2 = np.zeros((D, H))
    for h in range(H):
        A2[:, h] = scale * (Wk[h * DH:(h + 1) * DH, :].T @ q[h])
    U = 4.5 * float(np.linalg.norm(A2, axis=0).max())
    # a2pad: 4 placement variants; variant w (cols 32w..32w+32) has A2 at
    # within-block cols [8w, 8w+8), zeros elsewhere.
    a2p = np.zeros((D, 128))
    for w in range(4):
        a2p[:, 32 * w + 8 * w:32 * w + 8 * w + H] = A2
    # exp bias patterns: live slots get -U, dead rows -50 (exp -> fp16 zero).
    # half-group b even: tiles j<2 -> slot 0, j>=2 -> slot 1;
    # b odd: slots 2 and 3.  Row = 32j + 8*slot + h.
    ebias = np.full((128, 2), -50.0)
    for col, (w01, w23) in enumerate(((0, 1), (2, 3))):
        for j in range(4):
            w = w01 if j < 2 else w23
            ebias[32 * j + 8 * w:32 * j + 8 * w + H, col] = -U
    # ch layout: [c_in_half, (h, c_half, d_half, d_in_half)]
    ch = np.zeros((128, 32 * 128), np.float64)
    for h in range(H):
        CH_h = Wv[h * DH:(h + 1) * DH, :].T @ Wout[:, h * DH:(h + 1) * DH].T
        for c in range(2):
            for dh in range(2):
                blk = CH_h[c * 128:(c + 1) * 128, dh * 128:(dh + 1) * 128]
                ch[:, ((h * 2 + c) * 2 + dh) * 128:
                   ((h * 2 + c) * 2 + dh) * 128 + 128] = blk
    bout = np.asarray(out_proj_bias, np.float64)
    bout2 = np.stack([bout[:128], bout[128:]], axis=1)  # [128, 2]
    return a2p, ebias, U, ch, bout2


def _make_in_maps(x, a2p, ebias, ch, bout2, xnch=16):
    a2_16 = a2p.astype(np.float16)
    ebias_32 = ebias.astype(np.float32)
    ch_16 = ch.astype(np.float16)
    bout_32 = bout2.astype(np.float32)
    id16 = np.eye(128, dtype=np.float16)
    id32 = np.eye(128, dtype=np.float32)
    in_maps = []
    for c in range(N_CORES):
        xs = x[c * N_LOC:(c + 1) * N_LOC]
        x_nat = np.empty((N_LOC, D + 1), np.float16)
        x_nat[:, :D] = xs
        x_nat[:, D] = 1.0
        # pre-tile: row (128t+p) -> [chunk t//xnch, partition p, t%xnch, c]
        x_nat = np.ascontiguousarray(
            x_nat.reshape(NT // xnch, xnch, 128, D + 1).transpose(0, 2, 1, 3)
        ).reshape(NT // xnch, 128, xnch * (D + 1))
        x_tp = np.ascontiguousarray(xs.T.astype(np.float16))
        in_maps.append({
            "x_nat": x_nat, "x_t": x_tp, "a2": a2_16, "ebias": ebias_32,
            "ch": ch_16, "bout": bout_32, "id16": id16, "id32": id32,
        })
    return in_maps


def kernel(x, batch, query, in_proj_weight, in_proj_bias, out_proj_weight,
           out_proj_bias, num_heads, num_graphs):
    from concourse import bass_utils

    x = np.asarray(x, np.float32)
    batch = np.asarray(batch)
    assert x.shape == (N_TOTAL, D) and int(num_heads) == H
    assert int(num_graphs) == G_TOTAL
    expected = (np.arange(N_TOTAL, dtype=np.int64) * G_TOTAL) // N_TOTAL
    assert np.array_equal(batch.astype(np.int64), expected), \
        "kernel compiled for equal-sized segments"

    a2p, ebias, U, ch, bout2 = _host_fold(query, in_proj_weight,
                                          in_proj_bias, out_proj_weight,
                                          out_proj_bias)

    key = round(U, 9)
    if key not in _CACHE:
        _CACHE[key] = _build_program(U)
    nc = _CACHE[key]

    in_maps = _make_in_maps(x, a2p, ebias, ch, bout2)
    global _last_in_maps
    _last_in_maps = in_maps
    res = bass_utils.run_bass_kernel_spmd(nc, in_maps,
                                          core_ids=list(range(N_CORES)))
    out = np.concatenate([r["out"] for r in res.results], axis=0)
    return out.astype(np.float32)



# revision 4
# speedup vs baseline: 1.3283x; 1.3283x over previous
"""Trainium2 Bass kernel for single-query cross-attention pooling
(segment softmax over equal-sized node segments, 8-way node/graph sharded).

Self-contained: hardcodes N=262144, D=256, H=8, G=1024, 8 cores.

v2: fp8 (e3m4) for both HBM copies of x -> 17MB HBM traffic per core vs 33MB
for the fp16 baseline.  Precision budget (validated against the fp64
reference on the fixed seed-0 inputs): both x copies e3m4, a2/p/zn/ch fp16,
psum fp32 -> rel err ~1.4e-2 vs the 2e-2 gate.  The softmax weights p and
the folded projections stay fp16 because their quantization noise does NOT
average down relative to the signal (weighted-mean SNR floor).

Layout (all per core: 32768 nodes, 128 graphs, 256-node segments):
  scores: lhsT = xt8 tile [128 d-half, 128 nodes] (x as the matmul
          STATIONARY, FWL-eligible), rhs = a2 [128 d-half, 8 heads] fp16,
          2 chained MMs per tile -> psum s[128 nodes, 8] -- NATURAL node
          orientation, so no transposes of p are ever needed.
  exp:    one Act instr per 4-tile group writes p directly into the live
          8-col slots of a persistent, pre-zeroed [128, 128] buffer
          (strided 3D out AP).  Blocks j hold tile j's p at cols 32j..32j+8;
          cols +8..32 stay zero forever -> they are the zero-padding the
          pooling stationary needs.
  pool:   lhsT = p block [128 nodes, 32] fp16 (8 live cols), rhs = x_nat8
          tile [128 nodes, 257] e3m4 (col 256 = ones -> denominators),
          accumulated per graph into psum rows 32*(g%4) (tile_position).
          Window = 4 graphs; drain normalizes by 1/denom into zn fp16.
  tail:   zn window transposes + folded CH_h = Wv_h^T Wout_h^T projection
          (fp16), bias, final transpose -> out [128, 256] fp32.
"""

import numpy as np

N_TOTAL = 262144
D = 256
H = 8
DH = 32
G_TOTAL = 1024
N_CORES = 8

N_LOC = N_TOTAL // N_CORES          # 32768 nodes per core
G_LOC = G_TOTAL // N_CORES          # 128 graphs per core
SEG = N_TOTAL // G_TOTAL            # 256 nodes per graph
NT = N_LOC // 128                   # 256 node tiles per core
NB = NT // 4                        # 64 groups of 4 tiles
NW = NT // 8                        # 32 psum windows (4 graphs each)

XNCH = 16                           # x_nat tiles per DMA chunk
XT_N = 2048                         # xt nodes per DMA chunk

_CACHE = {}


def _build_program(U, repeats=1, tiny_out=False, null=False):
    import concourse.tile as tile
    from concourse import bacc, mybir
    from contextlib import ExitStack

    f16 = mybir.dt.float16
    f32 = mybir.dt.float32
    f8 = mybir.dt.float8e3

    nc = bacc.Bacc("TRN2", target_bir_lowering=False, debug=False,
                   num_devices=N_CORES)

    # xt[q, s, n] = x[n, q + 128 s]  (d on partitions, 2 half blocks)
    xt_d = nc.dram_tensor("xt", [128, 2, N_LOC], f8, kind="ExternalInput")
    # x_nat pre-tiled chunks: [chunk, partition, tile_in_chunk*(D+1)]
    xn_d = nc.dram_tensor("xn", [NT // XNCH, 128, XNCH * (D + 1)], f8,
                          kind="ExternalInput")
    a2_d = nc.dram_tensor("a2", [128, 2, H], f16, kind="ExternalInput")
    eb_d = nc.dram_tensor("eb", [128, 1], f32, kind="ExternalInput")
    ch_d = nc.dram_tensor("ch", [128, 32 * 128], f16, kind="ExternalInput")
    bout_d = nc.dram_tensor("bout", [128, 2], f32, kind="ExternalInput")
    id16_d = nc.dram_tensor("id16", [128, 128], f16, kind="ExternalInput")
    id32_d = nc.dram_tensor("id32", [128, 128], f32, kind="ExternalInput")
    out_shape = [1, 4] if tiny_out else [G_LOC, D]
    out_d = nc.dram_tensor("out", out_shape, f32, kind="ExternalOutput")

    Exp = mybir.ActivationFunctionType.Exp
    Copy = mybir.ActivationFunctionType.Copy
    Ident = mybir.ActivationFunctionType.Identity

    with tile.TileContext(nc) as tc, ExitStack() as ctx:
        consts = ctx.enter_context(tc.tile_pool(name="consts", bufs=1))
        xt_pool = ctx.enter_context(tc.tile_pool(name="xt_pool", bufs=3))
        xn_pool = ctx.enter_context(tc.tile_pool(name="xn_pool", bufs=3))
        zn_pool = ctx.enter_context(tc.tile_pool(name="zn_pool", bufs=2))
        small = ctx.enter_context(tc.tile_pool(name="small", bufs=4))
        tailsb = ctx.enter_context(tc.tile_pool(name="tailsb", bufs=2))
        sps = ctx.enter_context(tc.tile_pool(name="sps", bufs=2,
                                             space="PSUM"))
        zpps = ctx.enter_context(tc.tile_pool(name="zpps", bufs=2,
                                              space="PSUM"))
        smallps = ctx.enter_context(tc.tile_pool(name="smallps", bufs=2,
                                                 space="PSUM"))
        apsum = ctx.enter_context(tc.tile_pool(name="apsum", bufs=2,
                                               space="PSUM"))

        # ---- constants ----
        a2_sb = consts.tile([128, 2, H], f16)
        nc.gpsimd.dma_start(out=a2_sb, in_=a2_d.ap())
        eb_sb = consts.tile([128, 1], f32)
        nc.gpsimd.dma_start(out=eb_sb, in_=eb_d.ap())
        ch_sb = consts.tile([128, 32 * 128], f16)
        nc.gpsimd.dma_start(out=ch_sb, in_=ch_d.ap())
        bout_sb = consts.tile([128, 2], f32)
        nc.gpsimd.dma_start(out=bout_sb, in_=bout_d.ap())
        id16 = consts.tile([128, 128], f16)
        nc.gpsimd.dma_start(out=id16, in_=id16_d.ap())
        id32 = consts.tile([128, 128], f32)
        nc.gpsimd.dma_start(out=id32, in_=id32_d.ap())
        # persistent p buffers, one per group parity; cols 32j..32j+8 of
        # parity b%2 hold tile (4b+j)'s p, the rest stay zero forever
        pbig = [consts.tile([128, 128], f16, name=f"pbig{i}")
                for i in range(2)]
        nc.vector.memset(pbig[0], 0.0)
        nc.vector.memset(pbig[1], 0.0)

        def emit_body():
            state = {}
            znT = [tailsb.tile([128, NW * 128], f16, name=f"znT{c}",
                               tag=f"znT{c}") for c in range(2)]
            xt_c = xn_c = zp = None

            def phase_a(b):
                nonlocal xt_c, xn_c
                t0 = 4 * b
                n0 = 512 * b
                if n0 % XT_N == 0:
                    xt_c = xt_pool.tile([128, 2, XT_N], f8, name="xt")
                    nc.sync.dma_start(
                        out=xt_c, in_=xt_d.ap()[:, :, n0:n0 + XT_N])
                if t0 % XNCH == 0:
                    xn_c = xn_pool.tile([128, XNCH, D + 1], f8, name="xn")
                    nc.scalar.dma_start(
                        out=xn_c.rearrange("p k c -> p (k c)"),
                        in_=xn_d.ap()[t0 // XNCH])
                sp = sps.tile([128, 32], f32, name="sp", tag="sps")
                off = n0 % XT_N
                for j in range(4):
                    for s in range(2):
                        nc.tensor.matmul(
                            sp[:, 8 * j:8 * j + 8],
                            lhsT=xt_c[:, s, off + 128 * j:off + 128 * j
                                      + 128],
                            rhs=a2_sb[:, s, :],
                            start=(s == 0), stop=(s == 1))
                pb = pbig[b % 2]
                nc.scalar.activation(
                    out=pb.rearrange("p (j c) -> p j c", j=4)[:, :, 0:H],
                    in_=sp.rearrange("p (j h) -> p j h", j=4),
                    func=Exp, bias=eb_sb, scale=1.0)
                state[b] = (pb, xn_c)

            def phase_b(b):
                nonlocal zp
                pb, xn_b = state.pop(b)
                t0 = 4 * b
                for j in range(4):
                    t = t0 + j
                    g = t // 2
                    if t % 8 == 0:
                        zp = zpps.tile([128, D + 1], f32, name="zp")
                    strip = g % 4
                    nc.tensor.matmul(
                        zp[32 * strip:32 * strip + 32, :],
                        lhsT=pb[:, 32 * j:32 * j + 32],
                        rhs=xn_b[:, t % XNCH, :],
                        start=(t % 2 == 0), stop=(t % 2 == 1),
                        tile_position=(0, 32 * strip))
                if t0 % 8 == 4:     # window of 4 graphs complete
                    k = t0 // 8
                    den = small.tile([128, 1], f32, name="den")
                    nc.vector.tensor_scalar_add(den, zp[:, D:D + 1], 1e-20)
                    rd = small.tile([128, 1], f32, name="rd")
                    nc.vector.reciprocal(rd, den)
                    zn = zn_pool.tile([128, D], f16, name="zn")
                    nc.scalar.activation(out=zn, in_=zp[:, 0:D], func=Copy,
                                         bias=0.0, scale=rd)
                    for c in range(2):
                        tpz = smallps.tile([128, 128], f16, name="tpz",
                                           tag="tp")
                        nc.tensor.transpose(
                            tpz, zn[:, 128 * c:128 * (c + 1)], id16)
                        nc.vector.tensor_copy(
                            znT[c][:, 128 * k:128 * (k + 1)], tpz)

            for b in range(NB + 1):
                if b < NB:
                    phase_a(b)
                if b >= 1:
                    phase_b(b - 1)

            # ---- tail: project with folded CH, add bias ----
            outsb = tailsb.tile([128, D], f32, name="outsb")
            for dh in range(2):
                ap_ps = apsum.tile([128, 128], f32, name="ap_ps")
                for h in range(H):
                    for c in range(2):
                        w = ch_sb[:, ((h * 2 + c) * 2 + dh) * 128:
                                  ((h * 2 + c) * 2 + dh) * 128 + 128]
                        rhs = znT[c].rearrange(
                            "p (k v h2) -> p h2 (k v)", v=4, h2=32)[:, h, :]
                        nc.tensor.matmul(ap_ps, lhsT=w, rhs=rhs,
                                         start=(h == 0 and c == 0),
                                         stop=(h == H - 1 and c == 1))
                atb = tailsb.tile([128, 128], f32, name=f"atb{dh}")
                nc.scalar.activation(out=atb, in_=ap_ps, func=Ident,
                                     bias=bout_sb[:, dh:dh + 1], scale=1.0)
                tp2 = apsum.tile([128, 128], f32, name="ap_ps")
                nc.tensor.transpose(tp2, atb, id32)
                nc.vector.tensor_copy(outsb[:, 128 * dh:128 * (dh + 1)], tp2)

            if tiny_out:
                nc.sync.dma_start(out=out_d.ap(), in_=outsb[0:1, 0:4])
            else:
                nc.sync.dma_start(out=out_d.ap(), in_=outsb)

        if null:
            # transport-baseline NEFF: consts + output DMA only
            nc.sync.dma_start(out=out_d.ap()[0:1, 0:4], in_=id32[0:1, 0:4])
        else:
            for _rep in range(repeats):
                emit_body()

    nc.compile()
    return nc


def _host_fold(query, in_proj_weight, in_proj_bias, out_proj_weight,
               out_proj_bias):
    W = np.asarray(in_proj_weight, np.float64)
    Wq, Wk, Wv = W[:D], W[D:2 * D], W[2 * D:]
    b = np.asarray(in_proj_bias, np.float64)
    bq = b[:D]
    Wout = np.asarray(out_proj_weight, np.float64)
    scale = 1.0 / np.sqrt(DH)
    q = (np.asarray(query, np.float64).reshape(D) @ Wq.T + bq).reshape(H, DH)
    A2 = np.zeros((D, H))
    for h in range(H):
        A2[:, h] = scale * (Wk[h * DH:(h + 1) * DH, :].T @ q[h])
    U = 4.5 * float(np.linalg.norm(A2, axis=0).max())
    # a2[q, s, h] = A2[q + 128 s, h]
    a2 = np.ascontiguousarray(
        A2.reshape(2, 128, H).transpose(1, 0, 2))
    # ch layout: [c_in_half, (h, c_half, d_half, d_in_half)]
    ch = np.zeros((128, 32 * 128), np.float64)
    for h in range(H):
        CH_h = Wv[h * DH:(h + 1) * DH, :].T @ Wout[:, h * DH:(h + 1) * DH].T
        for c in range(2):
            for dh in range(2):
                blk = CH_h[c * 128:(c + 1) * 128, dh * 128:(dh + 1) * 128]
                ch[:, ((h * 2 + c) * 2 + dh) * 128:
                   ((h * 2 + c) * 2 + dh) * 128 + 128] = blk
    bout = np.asarray(out_proj_bias, np.float64)
    bout2 = np.stack([bout[:128], bout[128:]], axis=1)  # [128, 2]
    return a2, U, ch, bout2


def _make_in_maps(x, a2, U, ch, bout2):
    import ml_dtypes
    f8 = ml_dtypes.float8_e3m4

    a2_16 = a2.astype(np.float16)
    eb_32 = np.full((128, 1), -U, np.float32)
    ch_16 = ch.astype(np.float16)
    bout_32 = bout2.astype(np.float32)
    id16 = np.eye(128, dtype=np.float16)
    id32 = np.eye(128, dtype=np.float32)
    in_maps = []
    for c in range(N_CORES):
        xs = np.clip(x[c * N_LOC:(c + 1) * N_LOC], -15.5, 15.5)
        # xt[q, s, n] = x[n, q + 128 s]
        xt8 = np.ascontiguousarray(
            xs.T.reshape(2, 128, N_LOC).transpose(1, 0, 2)).astype(f8)
        x_nat = np.empty((N_LOC, D + 1), np.float32)
        x_nat[:, :D] = xs
        x_nat[:, D] = 1.0
        xn8 = np.ascontiguousarray(
            x_nat.reshape(NT // XNCH, XNCH, 128, D + 1)
            .transpose(0, 2, 1, 3)).astype(f8).reshape(
                NT // XNCH, 128, XNCH * (D + 1))
        in_maps.append({
            "xt": xt8, "xn": xn8, "a2": a2_16, "eb": eb_32,
            "ch": ch_16, "bout": bout_32, "id16": id16, "id32": id32,
        })
    return in_maps


def kernel(x, batch, query, in_proj_weight, in_proj_bias, out_proj_weight,
           out_proj_bias, num_heads, num_graphs):
    from concourse import bass_utils

    x = np.asarray(x, np.float32)
    batch = np.asarray(batch)
    assert x.shape == (N_TOTAL, D) and int(num_heads) == H
    assert int(num_graphs) == G_TOTAL
    expected = (np.arange(N_TOTAL, dtype=np.int64) * G_TOTAL) // N_TOTAL
    assert np.array_equal(batch.astype(np.int64), expected), \
        "kernel compiled for equal-sized segments"

    a2, U, ch, bout2 = _host_fold(query, in_proj_weight, in_proj_bias,
                                  out_proj_weight, out_proj_bias)

    key = round(U, 9)
    if key not in _CACHE:
        _CACHE[key] = _build_program(U)
    nc = _CACHE[key]

    in_maps = _make_in_maps(x, a2, U, ch, bout2)
    res = bass_utils.run_bass_kernel_spmd(nc, in_maps,
                                          core_ids=list(range(N_CORES)))
    out = np.concatenate([r["out"] for r in res.results], axis=0)
    return out.astype(np.float32)
